# revision 1
# baseline (speedup 1.0000x reference)
"""Trainium2 Bass kernel for batched two-layer-MLP attention.

Reference semantics (per batch b):
    x  = sequence[:, b, :]                        # [S, D]
    K  = tanh(tanh(x @ Kw1.T) @ Kw2.T)
    Q  = tanh(tanh(x @ Qw1.T) @ Qw2.T)
    W  = softmax(K @ Q.T / sqrt(D), axis=-1)      # [S, S]
    out[:, b, :] = W @ x

Sharding: data-parallel over batch (B=8 -> 8 NeuronCores), weights replicated.
Compute in bf16 on the TensorEngine (fp32 PSUM accumulation); softmax in fp32.

Layout strategy per core:
  - xt = x.T  [D, S]  (bf16, host-pretransposed)  -> MLP moving operand
  - weights pre-transposed to [d_in, d_out] so they serve directly as lhsT
  - MLP outputs stay transposed: Kt, Qt in [D, S]
  - scores SC[s, t] = sum_d Kt[d,s] * Qt[d,t]: lhsT=Kt tile, rhs=Qt -> natural
  - softmax along free axis (t); exp's accum_out gives the row sums for free
  - each exp(SC) row-block is transposed with ONE xbar DMA-transpose (bf16)
    into lhsT layout for attended = Wt.T @ x with rhs = xn [S, D]
  - 1/rowsum is folded into the PSUM->SBUF copy of the output (per-partition
    activation scale), so the big W matrix is never normalized.

Scheduling tricks: HAM warmup matmuls fill the initial DMA wait; first-layer
inputs are k/j-split into separate tiles across both HWDGE rings so the first
psum group starts after ~640KB of DMA; phase B's scores PSUM pool is opened
early so its banks don't overlap phase A's (no released-pool dependency);
phase B is software-pipelined one row-block ahead (attended(i-1) emitted after
scores(i)) so the PE never waits on the transpose.
"""

import numpy as np
import ml_dtypes

import concourse.bacc as bacc
import concourse.tile as tile
from concourse import mybir
from concourse.bass_utils import run_bass_kernel_spmd

P = 128          # partitions
S = 2048         # sequence length
D = 1024         # model dim
B = 8            # batch (one per core)
ST = S // P      # 16 s-tiles
DT = D // P      # 8 d-tiles
NF = 512         # psum free width (one bank of fp32)
SN = S // NF     # 4 score free-chunks
DN = D // NF     # 2 output free-chunks
BF = mybir.dt.bfloat16
F32 = mybir.dt.float32
SCALE = 1.0 / np.sqrt(np.float32(D))

AX = mybir.AxisListType.X
AF = mybir.ActivationFunctionType


def build_nc():
    nc = bacc.Bacc("TRN2", target_bir_lowering=False)

    xt_d = nc.dram_tensor("xt", [D, S], BF, kind="ExternalInput")
    xn_d = nc.dram_tensor("xn", [S, D], BF, kind="ExternalInput")
    # head tensor: x.T's n=0 chunk pre-packed [p, kh, k%, s] so each k-half
    # loads with one fully-contiguous DMA (first matmul fires earliest)
    xh_d = nc.dram_tensor("xh", [P, 2, DT // 2, NF], BF, kind="ExternalInput")
    # weights pre-arranged on the host to [p, j, k, c] so each j-block loads
    # with one partition-contiguous DMA (2KB/partition rows)
    WSHAPE = [P, DT, DT, P]
    wk1_d = nc.dram_tensor("wk1", WSHAPE, BF, kind="ExternalInput")
    wk2_d = nc.dram_tensor("wk2", WSHAPE, BF, kind="ExternalInput")
    wq1_d = nc.dram_tensor("wq1", WSHAPE, BF, kind="ExternalInput")
    wq2_d = nc.dram_tensor("wq2", WSHAPE, BF, kind="ExternalInput")
    out_d = nc.dram_tensor("out", [S, D], F32, kind="ExternalOutput")

    from contextlib import ExitStack

    with tile.TileContext(nc) as tc, ExitStack() as ctx:
        # ---- persistent SBUF arrays (live across both phases) ----
        pers = ctx.enter_context(tc.tile_pool(name="pers", bufs=1))
        xn_sb = pers.tile([P, ST, D], BF)     # x normal: [t-part, t-tile, d]
        kt_sb = pers.tile([P, DT, S], BF)     # K.T: [d-part, d-tile, s]
        # Q.T split per n-chunk so phase B's first scores don't wait on the
        # whole tensor's last tanh
        qt_n = [pers.tile([P, DT, NF], BF, tag=f"qt{n}", name=f"qt{n}")
                for n in range(SN)]

        # scores PSUM pool opened before phase A so it gets banks disjoint
        # from the MLP pool - phase B's first matmul then has no released-pool
        # overlap dependency on phase A's tail
        psc = ctx.enter_context(tc.tile_pool(name="psum_sc", bufs=3, space="PSUM"))

        # ---- phase A: the four MLP layers ----
        with tc.tile_pool(name="phase_a", bufs=1) as pa, \
             tc.tile_pool(name="wpool", bufs=2) as wp, \
             tc.tile_pool(name="psum_mlp", bufs=4, space="PSUM") as pm:
            # x.T split into per-n-chunk tiles so the first psum row's matmuls
            # only wait on the 1MB slice they read, not the whole 4MB array;
            # the n=0 chunk is additionally k-halved for an even earlier start
            KH = DT // 2
            xt_f = [pa.tile([P, KH, NF], BF, tag=f"xtf{h}", name=f"xtf{h}")
                    for h in range(2)]
            xt_n = [pa.tile([P, DT, NF], BF, tag=f"xt{n}", name=f"xt{n}")
                    for n in range(1, SN)]

            def xt_slice(n, k):
                if n == 0:
                    return xt_f[k // KH][:, k % KH, :]
                return xt_n[n - 1][:, k, :]

            h1_sb = pa.tile([P, DT, S], BF)   # hidden activations (reused K/Q)

            # HAM warmup: throwaway matmuls while the first input DMAs are in
            # flight, so the real matmuls start at 2.4GHz
            warm_sb = pa.tile([P, NF], BF)
            nc.vector.memset(warm_sb, 0.0)
            warm_ps = pm.tile([P, NF], F32, tag="warm", bufs=1)
            NWARM = 13
            for i in range(NWARM):
                nc.tensor.matmul(warm_ps, warm_sb[:, 0:P], warm_sb,
                                 start=(i == 0), stop=(i == NWARM - 1))

            def mlp_layer(src, w_dram, dst, xdma=None, first=False):
                # dst[j, s] = tanh(sum_k w[k, j].T @ src[k, s]) ; all transposed layout
                # one tile + one DMA per j-block so dep granularity is per-j.
                # For the first layer, j0/xt-n0 are further k-split in half so
                # the very first psum half-group starts after ~640KB of DMA;
                # xt goes on the SP ring, weights on the ACT ring, so neither
                # stream queues behind the other.
                xt_r = xt_d.rearrange("(k p) s -> p k s", p=P)
                if first:
                    w_jf = [wp.tile([P, KH, P], BF, tag=f"wf{h}", name=f"wf{h}")
                            for h in range(2)]
                    w_j = [wp.tile([P, DT, P], BF, tag=f"w{j}", name=f"w{j}")
                           for j in range(1, DT)]
                    for h in range(2):
                        nc.sync.dma_start(out=xt_f[h], in_=xh_d[:, h, :, :])
                        nc.scalar.dma_start(
                            out=w_jf[h], in_=w_dram[:, 0, h * KH:(h + 1) * KH, :])
                    for j in range(1, DT):
                        nc.scalar.dma_start(out=w_j[j - 1], in_=w_dram[:, j, :, :])
                    for n in range(1, SN):
                        nc.sync.dma_start(
                            out=xt_n[n - 1], in_=xt_r[:, :, n * NF:(n + 1) * NF])

                    def lhs_sl(j, k):
                        return (w_jf[k // KH][:, k % KH, :] if j == 0
                                else w_j[j - 1][:, k, :])
                else:
                    w_j = [wp.tile([P, DT, P], BF, tag=f"w{j}", name=f"w{j}")
                           for j in range(DT)]
                    for j in range(DT):
                        nc.sync.dma_start(out=w_j[j], in_=w_dram[:, j, :, :])
                    if xdma is not None:
                        xdma()

                    def lhs_sl(j, k):
                        return w_j[j][:, k, :]

                def rhs_sl(n, k):
                    return (xt_slice(n, k) if src is None
                            else src[:, k, n * NF:(n + 1) * NF])

                loop = ([(j, n) for n in range(SN) for j in range(DT)] if first
                        else [(j, n) for j in range(DT) for n in range(SN)])
                for j, n in loop:
                    ps = pm.tile([P, NF], F32, tag="mlp")
                    for k in range(DT):
                        nc.tensor.matmul(
                            ps,
                            lhs_sl(j, k),
                            rhs_sl(n, k),
                            start=(k == 0),
                            stop=(k == DT - 1),
                        )
                    dslice = (dst[n][:, j, :] if isinstance(dst, list)
                              else dst[:, j, n * NF:(n + 1) * NF])
                    nc.scalar.activation(out=dslice, in_=ps, func=AF.Tanh)

            def load_xn():
                xn_r = xn_d.rearrange("(t p) d -> p t d", p=P)
                for t in range(0, ST, 4):
                    nc.sync.dma_start(out=xn_sb[:, t:t + 4, :],
                                      in_=xn_r[:, t:t + 4, :])

            mlp_layer(None, wk1_d, h1_sb, first=True)
            mlp_layer(h1_sb, wk2_d, kt_sb)
            mlp_layer(None, wq1_d, h1_sb, xdma=load_xn)
            mlp_layer(h1_sb, wq2_d, qt_n)

        # ---- phase B: scores -> softmax -> transpose -> attended ----
        with tc.tile_pool(name="wexp", bufs=2) as wexp_pool, \
             tc.tile_pool(name="wtT", bufs=2) as wtT_pool, \
             tc.tile_pool(name="sums", bufs=4) as sums_pool, \
             tc.tile_pool(name="outst", bufs=2) as out_pool, \
             tc.tile_pool(name="psum_at", bufs=3, space="PSUM") as pat:

            def scores_softmax_transpose(i):
                """Row-block i of exp(scores) plus its reciprocal row sums,
                transposed into lhsT layout for the attended matmul."""
                wexp = wexp_pool.tile([P, S], BF, tag="wexp")
                sums = sums_pool.tile([P, SN], F32, tag="sums")
                for n in range(SN):
                    ps = psc.tile([P, NF], F32, tag="sc")
                    for k in range(DT):
                        nc.tensor.matmul(
                            ps,
                            kt_sb[:, k, i * P:(i + 1) * P],
                            qt_n[n][:, k, :],
                            start=(k == 0),
                            stop=(k == DT - 1),
                        )
                    # scores are bounded (|sc/32| < ~3): exp without max-shift
                    nc.scalar.activation(
                        out=wexp[:, n * NF:(n + 1) * NF],
                        in_=ps,
                        func=AF.Exp,
                        scale=float(SCALE),
                        accum_out=sums[:, n:n + 1],
                    )
                rcp = sums_pool.tile([P, 1], F32, tag="rcp")
                nc.vector.reduce_sum(rcp, sums, axis=AX)
                nc.vector.reciprocal(rcp, rcp)
                # one xbar transpose of the whole row-block:
                #   wtT[p, t, c] = wexp[c, t*128 + p]
                wtT = wtT_pool.tile([P, ST, P], BF, tag="wtT")
                nc.scalar.dma_start_transpose(out=wtT, in_=wexp)
                return wtT, rcp

            def attended(i, wtT, rcp, last=False):
                outst = out_pool.tile([P, D], F32, tag="outst")
                for n in range(DN):
                    ps = pat.tile([P, NF], F32, tag="at")
                    for t in range(ST):
                        nc.tensor.matmul(
                            ps,
                            wtT[:, t, :],
                            xn_sb[:, t, n * NF:(n + 1) * NF],
                            start=(t == 0),
                            stop=(t == ST - 1),
                        )
                    # fold the softmax normalization into the PSUM->SBUF copy
                    nc.scalar.mul(outst[:, n * NF:(n + 1) * NF], ps, rcp)
                    nc.sync.dma_start(
                        out=out_d[i * P:(i + 1) * P, n * NF:(n + 1) * NF],
                        in_=outst[:, n * NF:(n + 1) * NF],
                    )

            # software-pipelined: attended(i-1) is emitted after scores(i) so
            # the PE never waits on the DVE transpose copies
            prev = None
            for i in range(ST):
                cur = scores_softmax_transpose(i)
                if prev is not None:
                    attended(i - 1, *prev)
                prev = cur
            attended(ST - 1, *prev, last=True)

    nc.compile()
    return nc


_NC = None


def _get_nc():
    global _NC
    if _NC is None:
        _NC = build_nc()
    return _NC


def _prep_w(w):
    """[d_out, d_in] f32 -> [p, j, k, c] bf16 of w.T (k,p index d_in; j,c d_out)."""
    wt = np.asarray(w).T.reshape(DT, P, DT, P).transpose(1, 2, 0, 3)
    return np.ascontiguousarray(wt).astype(ml_dtypes.bfloat16)


def make_in_maps(sequence, Kw1, Kw2, Qw1, Qw2):
    bf16 = ml_dtypes.bfloat16
    seq = np.ascontiguousarray(np.transpose(np.asarray(sequence), (1, 0, 2)))  # [B, S, D]
    ws = {"wk1": _prep_w(Kw1), "wk2": _prep_w(Kw2),
          "wq1": _prep_w(Qw1), "wq2": _prep_w(Qw2)}
    in_maps = []
    for b in range(B):
        xb = seq[b]
        xt = np.ascontiguousarray(xb.T).astype(bf16)
        # [P, 2, KH, NF]: xh[p, h, q, s] = xt[(h*KH + q)*P + p, s] for s < NF
        xh = np.ascontiguousarray(
            xt[:, 0:NF].reshape(2, DT // 2, P, NF).transpose(2, 0, 1, 3))
        m = {"xn": xb.astype(bf16), "xt": xt, "xh": xh}
        m.update(ws)
        in_maps.append(m)
    return in_maps


def kernel(sequence, Kw1, Kw2, Qw1, Qw2):
    nc = _get_nc()
    in_maps = make_in_maps(sequence, Kw1, Kw2, Qw1, Qw2)
    res = run_bass_kernel_spmd(nc, in_maps, core_ids=list(range(B)))
    out = np.stack([res.results[b]["out"] for b in range(B)], axis=1)
    return out.astype(np.float32)

